# revision 13
# baseline (speedup 1.0000x reference)
"""CBIndirectionLookup Trainium2 kernel (v15: 2-bit selector fold, 64-wide
one-hot, row-paired broadcast/one-hot/gather, batched recombine).

Problem: x [N=2097152, 8] int32 bit-vectors; patterns [256, 8] (unique bit rows);
results [256, 4] int32. Output: results[argmax(all(x==patterns))] -> [N, 4] int32.

Math: c8 = sum_j x_j 2^j = c6 + 64 b6 + 128 b7.  With Moebius tables over
(b6, b7):
    B   = T[c6],             D6  = T[c6+64]  - T[c6]
    D7  = T[c6+128] - T[c6], D67 = T[c6+192] - T[c6+128] - T[c6+64] + T[c6]
    y = B + b6*D6 + b7*D7 + (b6 b7)*D67        (all |.| <= 1998: fp16-exact)

Per core (262144 elems; 4 groups of 4 blocks; block = [128p, 128i]):
 1. DVE Horner on uint8 bytes -> c8 int16 -> c6 fp32; selector tiles
    s6w/s7w/s67w [128, 2048] fp16 (w-replicated x4).
 2. PE transpose c6 -> ct [128, 512] fp16 rows per group (via psum + ACT copy).
 3. Per row-PAIR (64/group): one matmul with lhsT=sel2 [2,128] broadcasts row
    2j to partitions 0-63 and row 2j+1 to 64-127; ACT-copy psum->sbuf fp16;
    one is_equal vs iota64 -> one-hot pair [128, 512] fp16.
 4. Gather: per block bb, matmul(lhsT=onehot chunk [128,128], rhs=luts2
    [128, 32]) -> psum [128elem, 32] = (tbl, r, w) fetches, element-major.
 5. Every 8 pairs: 6 batched DVE ops recombine y = B + s6 D6 + s7 D7 + s67 D67
    -> int32, one DMA out per 2-bank drain.
"""
import sys
sys.path.insert(0, "/opt/trn_rl_repo")

import numpy as np

N = 2_097_152
W_IN = 8
W_OUT = 4
N_CORES = 8
N_LOC = N // N_CORES            # 262144 elements per core
BLK_ELEMS = 16384               # elements per block: [128, 128] layout
N_BLKS = N_LOC // BLK_ELEMS     # 16
EPP = BLK_ELEMS // 128          # 128 elements per partition per block
GROUP_BLKS = 4                  # blocks per code-transpose group
N_GROUPS = N_BLKS // GROUP_BLKS
PAIRS = 64                      # row pairs per group
DRAIN_PAIRS = 8                 # pairs per psum drain (2 banks)


def _build_luts(patterns: np.ndarray, results: np.ndarray):
    """Fold tables into luts2 [128, 32] fp16: block-diag 2x luts16 [64, 16].

    luts16[c6, tbl*4+w] with tbl in {B, D6, D7, D67}; luts2 col layout
    tbl*8 + r*4 + w (r = which row of the pair / diag block)."""
    pat2idx = {}
    for p in range(patterns.shape[0]):
        pat2idx[tuple(int(v) for v in patterns[p])] = p
    T = np.zeros((256, W_OUT), np.float64)
    for c in range(256):
        bits = [(c >> j) & 1 for j in range(8)]
        T[c] = results[pat2idx[tuple(bits)]]
    c6 = np.arange(64)
    B = T[c6]
    D6 = T[c6 + 64] - T[c6]
    D7 = T[c6 + 128] - T[c6]
    D67 = T[c6 + 192] - T[c6 + 128] - T[c6 + 64] + T[c6]
    luts16 = np.stack([B, D6, D7, D67], axis=1)        # [64, 4tbl, 4w]
    luts2 = np.zeros((128, 4, 2, 4), np.float64)       # [row, tbl, r, w]
    luts2[0:64, :, 0, :] = luts16
    luts2[64:128, :, 1, :] = luts16
    out = luts2.reshape(128, 32).astype(np.float16)
    assert np.array_equal(out.astype(np.float64), luts2.reshape(128, 32))
    return out


def _build_kernel():
    import concourse.bass as bass
    import concourse.bacc as bacc
    import concourse.tile as tile
    from concourse import mybir

    nc = bacc.Bacc("TRN2", target_bir_lowering=False, debug=False,
                   num_devices=N_CORES)
    dt = mybir.dt
    x = nc.dram_tensor("x", [128, N_LOC // 128 * W_IN], dt.uint8,
                       kind="ExternalInput").ap()
    luts = nc.dram_tensor("luts", [128, 32], dt.float16,
                          kind="ExternalInput").ap()
    sel2 = nc.dram_tensor("sel2", [2, 128], dt.float16,
                          kind="ExternalInput").ap()
    y = nc.dram_tensor("y", [128, N_LOC // 128 * W_OUT], dt.int32,
                       kind="ExternalOutput").ap()
    ct_scratch = nc.dram_tensor("ct_scratch", [2, 128, GROUP_BLKS * 128],
                                dt.float16).ap()

    with tile.TileContext(nc) as tc:
        with (
            tc.tile_pool(name="const", bufs=1) as constp,
            tc.tile_pool(name="xin", bufs=3) as xinp,
            tc.tile_pool(name="scan", bufs=3) as scanp,
            tc.tile_pool(name="sel", bufs=2) as selp,
            tc.tile_pool(name="ct", bufs=2) as ctp,
            tc.tile_pool(name="stg", bufs=2) as stgp,
            tc.tile_pool(name="bc", bufs=4) as bcp,
            tc.tile_pool(name="w", bufs=4) as wp,
            tc.tile_pool(name="scr", bufs=2) as scrp,
            tc.tile_pool(name="out", bufs=3) as outp,
            tc.tile_pool(name="pct", bufs=1, space="PSUM") as pctp,
            tc.tile_pool(name="pbc", bufs=2, space="PSUM") as pbcp,
            tc.tile_pool(name="pgd", bufs=2, space="PSUM") as pgdp,
        ):
            t_luts = constp.tile([128, 32], dt.float16)
            nc.sync.dma_start(out=t_luts[:], in_=luts[:])
            t_iota32 = constp.tile([128, 1], dt.int32)
            nc.gpsimd.iota(t_iota32[:], pattern=[[0, 1]], base=0,
                           channel_multiplier=1)
            t_iota64i = constp.tile([128, 1], dt.int32)
            nc.vector.tensor_scalar(out=t_iota64i[:], in0=t_iota32[:],
                                    scalar1=63, scalar2=None,
                                    op0=mybir.AluOpType.bitwise_and)
            t_iota64 = constp.tile([128, 1], dt.float32)
            nc.vector.tensor_copy(t_iota64[:], t_iota64i[:])
            t_ident = constp.tile([128, 128], dt.float32)
            from concourse.masks import make_identity
            make_identity(nc, t_ident[:])
            # sel2 [2, 128]: row0 -> partitions 0-63, row1 -> partitions 64-127
            t_sel2 = constp.tile([2, 128], dt.float16)
            nc.sync.dma_start(out=t_sel2[:], in_=sel2[:])

            for g in range(N_GROUPS):
                # --- scan + transpose + selector tiles for this group ---
                t_ct_ps = pctp.tile([128, GROUP_BLKS * 128], dt.float32)
                t_s6w = selp.tile([128, GROUP_BLKS * EPP * 4], dt.float16,
                                  tag="s6")
                t_s7w = selp.tile([128, GROUP_BLKS * EPP * 4], dt.float16,
                                  tag="s7")
                t_s67w = selp.tile([128, GROUP_BLKS * EPP * 4], dt.float16,
                                   tag="s67")
                for bb in range(GROUP_BLKS):
                    b = g * GROUP_BLKS + bb
                    t_x = xinp.tile([128, EPP * 8], dt.uint8)
                    nc.sync.dma_start(
                        out=t_x[:],
                        in_=x[:, b * EPP * 8:(b + 1) * EPP * 8])
                    x3 = t_x[:].rearrange("p (e k) -> p e k", k=8)
                    t_f1 = scanp.tile([128, EPP * 4], dt.int16, tag="f1")
                    f13 = t_f1[:].rearrange("p (e k) -> p e k", k=4)
                    nc.vector.scalar_tensor_tensor(
                        out=f13[:, :, :], in0=x3[:, :, 1::2], scalar=2.0,
                        in1=x3[:, :, 0::2], op0=mybir.AluOpType.mult,
                        op1=mybir.AluOpType.add)
                    t_f2 = scanp.tile([128, EPP * 2], dt.int16, tag="f2")
                    f23 = t_f2[:].rearrange("p (e k) -> p e k", k=2)
                    nc.vector.scalar_tensor_tensor(
                        out=f23[:, :, :], in0=f13[:, :, 1::2], scalar=4.0,
                        in1=f13[:, :, 0::2], op0=mybir.AluOpType.mult,
                        op1=mybir.AluOpType.add)
                    # f2[:,:,0] = bits0-3 value, f2[:,:,1] = bits4-7 value
                    t_c8 = scanp.tile([128, EPP], dt.int16, tag="c8")
                    nc.vector.scalar_tensor_tensor(
                        out=t_c8[:], in0=f23[:, :, 1], scalar=16.0,
                        in1=f23[:, :, 0], op0=mybir.AluOpType.mult,
                        op1=mybir.AluOpType.add)
                    t_c67 = scanp.tile([128, EPP], dt.int16, tag="c67")
                    nc.vector.scalar_tensor_tensor(
                        out=t_c67[:], in0=x3[:, :, 6], scalar=-64.0,
                        in1=t_c8[:], op0=mybir.AluOpType.mult,
                        op1=mybir.AluOpType.add)
                    t_c6 = scanp.tile([128, EPP], dt.float32, tag="c6")
                    nc.vector.scalar_tensor_tensor(
                        out=t_c6[:], in0=x3[:, :, 7], scalar=-128.0,
                        in1=t_c67[:], op0=mybir.AluOpType.mult,
                        op1=mybir.AluOpType.add)
                    # selector tiles (w-replicated x4): cols bb*512 + e*4 + w
                    def selview(t):
                        return t[:, bb * EPP * 4:(bb + 1) * EPP * 4]
                    def xbit(j):
                        v = x3[:, :, j]
                        return bass.AP(tensor=v.tensor, offset=v.offset,
                                       ap=v.ap + [[0, 4]])
                    nc.vector.tensor_scalar(
                        out=selview(t_s6w).rearrange("p (e k) -> p e k", k=4),
                        in0=xbit(6), scalar1=1.0, scalar2=None,
                        op0=mybir.AluOpType.mult)
                    nc.vector.tensor_scalar(
                        out=selview(t_s7w).rearrange("p (e k) -> p e k", k=4),
                        in0=xbit(7), scalar1=1.0, scalar2=None,
                        op0=mybir.AluOpType.mult)
                    nc.vector.tensor_tensor(
                        out=selview(t_s67w), in0=selview(t_s6w),
                        in1=selview(t_s7w), op=mybir.AluOpType.mult)
                    # transpose c6 into group psum columns
                    nc.tensor.transpose(
                        t_ct_ps[:, bb * 128:(bb + 1) * 128],
                        t_c6[:], t_ident[:])
                t_ct = ctp.tile([128, GROUP_BLKS * 128], dt.float16)
                nc.scalar.copy(t_ct[:], t_ct_ps[:])
                nc.sync.dma_start(out=ct_scratch[g % 2], in_=t_ct[:])

                # --- pair loop ---
                t_pg = None
                t_stage = None
                STG_PAIRS = 16
                for jp in range(PAIRS):
                    jd = jp % DRAIN_PAIRS
                    if jd == 0:
                        t_pg = pgdp.tile([128, DRAIN_PAIRS * 128], dt.float32)
                    if jp % STG_PAIRS == 0:
                        # load 16 pairs: stage[r, k*512+f] = ct[i0+2k+r, f]
                        t_stage = stgp.tile([2, STG_PAIRS * 512], dt.float16)
                        cs = ct_scratch[g % 2]
                        src = bass.AP(
                            tensor=cs.tensor,
                            offset=cs.offset + (2 * jp) * 512,
                            ap=[[512, 2], [1024, STG_PAIRS], [1, 512]])
                        nc.sync.dma_start(out=t_stage[:].rearrange(
                            "p (k f) -> p k f", f=512), in_=src)
                        # odd rows again at partition 0 for gpsimd bcast
                        t_stage1 = stgp.tile([1, STG_PAIRS * 512], dt.float16,
                                             tag="s1")
                        src1 = bass.AP(
                            tensor=cs.tensor,
                            offset=cs.offset + (2 * jp + 1) * 512,
                            ap=[[512, 1], [1024, STG_PAIRS], [1, 512]])
                        nc.sync.dma_start(out=t_stage1[:].rearrange(
                            "p (k f) -> p k f", f=512), in_=src1)
                    # broadcast pair of code rows
                    ks = (jp % STG_PAIRS) * 512
                    t_bc = bcp.tile([128, 512], dt.float16)
                    if jp % 4 == 2:
                        # gpsimd path: two full broadcasts, half is_eq each
                        t_bcB = bcp.tile([128, 512], dt.float16, tag="bcB")
                        nc.gpsimd.partition_broadcast(
                            t_bc[:], t_stage[0:1, ks:ks + 512])
                        nc.gpsimd.partition_broadcast(
                            t_bcB[:], t_stage1[0:1, ks:ks + 512])
                        t_w = wp.tile([128, 512], dt.float16)
                        nc.vector.tensor_scalar(
                            out=t_w[0:64, :], in0=t_bc[0:64, :],
                            scalar1=t_iota64[0:64], scalar2=None,
                            op0=mybir.AluOpType.is_equal)
                        nc.vector.tensor_scalar(
                            out=t_w[64:128, :], in0=t_bcB[64:128, :],
                            scalar1=t_iota64[64:128], scalar2=None,
                            op0=mybir.AluOpType.is_equal)
                    else:
                        t_bc_ps = pbcp.tile([128, 512], dt.float32)
                        nc.tensor.matmul(t_bc_ps[:], t_sel2[:],
                                         t_stage[0:2, ks:ks + 512],
                                         start=True, stop=True,
                                         tile_position=(0, 0))
                        if jp % 2 == 0:
                            nc.scalar.copy(t_bc[:], t_bc_ps[:])
                        else:
                            nc.vector.tensor_copy(t_bc[:], t_bc_ps[:])
                        t_w = wp.tile([128, 512], dt.float16)
                        nc.vector.tensor_scalar(
                            out=t_w[:], in0=t_bc[:], scalar1=t_iota64[:],
                            scalar2=None, op0=mybir.AluOpType.is_equal)
                    # gather: per block, one-hot chunk stationary x luts2
                    for bb in range(GROUP_BLKS):
                        nc.tensor.matmul(
                            t_pg[:, jd * 128 + bb * 32:jd * 128 + bb * 32 + 32],
                            t_w[:, bb * 128:(bb + 1) * 128],
                            t_luts[:],
                            start=True, stop=True)
                    if jd == DRAIN_PAIRS - 1:
                        # recombine 8 pairs: psum cols j*128 + bb*32 + tbl*8
                        #   + r*4 + w ; view dims (bb, j, rw)
                        i0 = (jp - jd) * 2      # first row index of drain
                        P = t_pg[:]
                        def tblview(t):
                            return bass.AP(
                                tensor=P.tensor, offset=P.offset + t * 8,
                                ap=[P.ap[0], [32, 4], [128, 8], [1, 8]])
                        def sview(t):
                            base = t[:]
                            return bass.AP(
                                tensor=base.tensor,
                                offset=base.offset + i0 * 4,
                                ap=[base.ap[0], [EPP * 4, 4], [8, 8], [1, 8]])
                        t_u1 = scrp.tile([128, 256], dt.float32, tag="u1")
                        t_u2 = scrp.tile([128, 256], dt.float32, tag="u2")
                        t_u3 = scrp.tile([128, 256], dt.float32, tag="u3")
                        scr3 = lambda t: t[:].rearrange(
                            "p (a b c) -> p a b c", a=4, b=8)
                        nc.vector.tensor_tensor(
                            out=scr3(t_u1), in0=tblview(1), in1=sview(t_s6w),
                            op=mybir.AluOpType.mult)
                        nc.vector.tensor_tensor(
                            out=scr3(t_u2), in0=tblview(2), in1=sview(t_s7w),
                            op=mybir.AluOpType.mult)
                        nc.vector.tensor_tensor(
                            out=scr3(t_u3), in0=tblview(3), in1=sview(t_s67w),
                            op=mybir.AluOpType.mult)
                        t_a1 = scrp.tile([128, 256], dt.float32, tag="a1")
                        nc.vector.tensor_tensor(
                            out=scr3(t_a1), in0=tblview(0), in1=scr3(t_u1),
                            op=mybir.AluOpType.add)
                        t_a2 = scrp.tile([128, 256], dt.float32, tag="a2")
                        nc.vector.tensor_tensor(
                            out=scr3(t_a2), in0=scr3(t_u2), in1=scr3(t_u3),
                            op=mybir.AluOpType.add)
                        t_o = outp.tile([128, 256], dt.int32)
                        nc.vector.tensor_tensor(
                            out=scr3(t_o), in0=scr3(t_a1), in1=scr3(t_a2),
                            op=mybir.AluOpType.add)
                        # dma out: o cols (bb, j, rw) -> y col b*512 + i0*4
                        #   + (j*8 + r*4 + w);  (j, rw) contiguous 64
                        yv = bass.AP(
                            tensor=y.tensor,
                            offset=y.offset + g * GROUP_BLKS * 512 + i0 * 4,
                            ap=[y.ap[0], [512, 4], [1, 64]])
                        ov = t_o[:].rearrange("p (a b) -> p a b", a=4)
                        nc.sync.dma_start(out=yv, in_=ov)
    nc.compile()
    return nc


_CACHE = {}


def kernel(x: np.ndarray, patterns: np.ndarray, results: np.ndarray) -> np.ndarray:
    import jax
    from jax.sharding import Mesh, PartitionSpec, NamedSharding
    from jax.experimental.shard_map import shard_map
    from concourse import mybir
    from concourse.bass2jax import (_bass_exec_p, install_neuronx_cc_hook,
                                    partition_id_tensor)

    x = np.asarray(x)
    patterns = np.asarray(patterns)
    results = np.asarray(results)
    rhs_luts = _build_luts(patterns, results)

    if "nc" not in _CACHE:
        _CACHE["nc"] = _build_kernel()
    nc = _CACHE["nc"]

    install_neuronx_cc_hook()
    partition_name = nc.partition_id_tensor.name if nc.partition_id_tensor else None
    in_names, out_names, out_avals, zero_outs = [], [], [], []
    for alloc in nc.m.functions[0].allocations:
        if not isinstance(alloc, mybir.MemoryLocationSet):
            continue
        name = alloc.memorylocations[0].name
        if alloc.kind == "ExternalInput":
            if name != partition_name:
                in_names.append(name)
        elif alloc.kind == "ExternalOutput":
            out_names.append(name)
            shape = tuple(alloc.tensor_shape)
            dtype = mybir.dt.np(alloc.dtype)
            out_avals.append(jax.core.ShapedArray(shape, dtype))
            zero_outs.append(np.zeros(shape, dtype))
    n_params = len(in_names)
    n_outs = len(out_avals)
    all_in_names = in_names + out_names + ([partition_name] if partition_name else [])

    def _body(*args):
        operands = list(args)
        if partition_name is not None:
            operands.append(partition_id_tensor())
        outs = _bass_exec_p.bind(
            *operands, out_avals=tuple(out_avals), in_names=tuple(all_in_names),
            out_names=tuple(out_names), lowering_input_output_aliases=(),
            sim_require_finite=False, sim_require_nnan=False, nc=nc)
        return tuple(outs)

    devices = jax.devices()[:N_CORES]
    mesh = Mesh(np.asarray(devices), ("core",))
    shard = NamedSharding(mesh, PartitionSpec("core"))
    fn = jax.jit(
        shard_map(_body, mesh=mesh,
                  in_specs=(PartitionSpec("core"),) * (n_params + n_outs),
                  out_specs=(PartitionSpec("core"),) * n_outs,
                  check_rep=False),
        keep_unused=True)

    # Per-core input planes; element n_loc = b*16384 + p*128 + i.
    # x values are 0/1 int32 little-endian: byte 0 of each word is the bit.
    xb = x.view(np.uint8).reshape(N, W_IN, 4)[:, :, 0]         # [N, 8] uint8
    xc = xb.reshape(N_CORES, N_BLKS, 128, EPP * W_IN)          # [c, b, p, 128*8]
    x_in = np.ascontiguousarray(xc.transpose(0, 2, 1, 3)).reshape(
        N_CORES * 128, N_BLKS * EPP * W_IN)
    luts_in = np.broadcast_to(rhs_luts, (N_CORES, 128, 32)).reshape(
        N_CORES * 128, 32)
    sel2_np = np.zeros((2, 128), np.float16)
    sel2_np[0, 0:64] = 1.0
    sel2_np[1, 64:128] = 1.0
    sel2_in = np.broadcast_to(sel2_np, (N_CORES, 2, 128)).reshape(N_CORES * 2, 128)
    arrays = {"x": x_in, "luts": np.ascontiguousarray(luts_in),
              "sel2": np.ascontiguousarray(sel2_in)}
    args = [jax.device_put(arrays[nm], shard) for nm in in_names]
    args += [jax.device_put(
        np.zeros((N_CORES * z.shape[0], *z.shape[1:]), z.dtype), shard)
        for z in zero_outs]
    out_arrs = fn(*args)
    yi = out_names.index("y")
    yv = np.asarray(out_arrs[yi]).reshape(N_CORES, 128, N_BLKS, EPP * W_OUT)
    # invert layout: [c, p, b, 128*4] -> [c, b, p, i, 4] -> n
    y_full = yv.transpose(0, 2, 1, 3).reshape(N, W_OUT)
    return y_full.astype(np.int32)


# revision 14
# speedup vs baseline: 1.2318x; 1.2318x over previous
"""CBIndirectionLookup Trainium2 kernel (v15: 2-bit selector fold, 64-wide
one-hot, row-paired broadcast/one-hot/gather, batched recombine).

Problem: x [N=2097152, 8] int32 bit-vectors; patterns [256, 8] (unique bit rows);
results [256, 4] int32. Output: results[argmax(all(x==patterns))] -> [N, 4] int32.

Math: c8 = sum_j x_j 2^j = c6 + 64 b6 + 128 b7.  With Moebius tables over
(b6, b7):
    B   = T[c6],             D6  = T[c6+64]  - T[c6]
    D7  = T[c6+128] - T[c6], D67 = T[c6+192] - T[c6+128] - T[c6+64] + T[c6]
    y = B + b6*D6 + b7*D7 + (b6 b7)*D67        (all |.| <= 1998: fp16-exact)

Per core (262144 elems; 4 groups of 4 blocks; block = [128p, 128i]):
 1. DVE Horner on uint8 bytes -> c8 int16 -> c6 fp32; selector tiles
    s6w/s7w/s67w [128, 2048] fp16 (w-replicated x4).
 2. PE transpose c6 -> ct [128, 512] fp16 rows per group (via psum + ACT copy).
 3. Per row-PAIR (64/group): one matmul with lhsT=sel2 [2,128] broadcasts row
    2j to partitions 0-63 and row 2j+1 to 64-127; ACT-copy psum->sbuf fp16;
    one is_equal vs iota64 -> one-hot pair [128, 512] fp16.
 4. Gather: per block bb, matmul(lhsT=onehot chunk [128,128], rhs=luts2
    [128, 32]) -> psum [128elem, 32] = (tbl, r, w) fetches, element-major.
 5. Every 8 pairs: 6 batched DVE ops recombine y = B + s6 D6 + s7 D7 + s67 D67
    -> int32, one DMA out per 2-bank drain.
"""
import sys
sys.path.insert(0, "/opt/trn_rl_repo")

import numpy as np

N = 2_097_152
W_IN = 8
W_OUT = 4
N_CORES = 8
N_LOC = N // N_CORES            # 262144 elements per core
BLK_ELEMS = 16384               # elements per block: [128, 128] layout
N_BLKS = N_LOC // BLK_ELEMS     # 16
EPP = BLK_ELEMS // 128          # 128 elements per partition per block
GROUP_BLKS = 4                  # blocks per code-transpose group
N_GROUPS = N_BLKS // GROUP_BLKS
PAIRS = 64                      # row pairs per group
DRAIN_PAIRS = 8                 # pairs per psum drain (2 banks)


def _build_luts(patterns: np.ndarray, results: np.ndarray):
    """Fold tables into luts2 [128, 32] fp16: block-diag 2x luts16 [64, 16].

    luts16[c6, tbl*4+w] with tbl in {B, D6, D7, D67}; luts2 col layout
    tbl*8 + r*4 + w (r = which row of the pair / diag block)."""
    pat2idx = {}
    for p in range(patterns.shape[0]):
        pat2idx[tuple(int(v) for v in patterns[p])] = p
    T = np.zeros((256, W_OUT), np.float64)
    for c in range(256):
        bits = [(c >> j) & 1 for j in range(8)]
        T[c] = results[pat2idx[tuple(bits)]]
    c6 = np.arange(64)
    B = T[c6]
    D6 = T[c6 + 64] - T[c6]
    D7 = T[c6 + 128] - T[c6]
    D67 = T[c6 + 192] - T[c6 + 128] - T[c6 + 64] + T[c6]
    luts16 = np.stack([B, D6, D7, D67], axis=1)        # [64, 4tbl, 4w]
    luts2 = np.zeros((128, 4, 2, 4), np.float64)       # [row, tbl, r, w]
    luts2[0:64, :, 0, :] = luts16
    luts2[64:128, :, 1, :] = luts16
    out = luts2.reshape(128, 32).astype(np.float16)
    assert np.array_equal(out.astype(np.float64), luts2.reshape(128, 32))
    return out


def _build_kernel():
    import concourse.bass as bass
    import concourse.bacc as bacc
    import concourse.tile as tile
    from concourse import mybir

    nc = bacc.Bacc("TRN2", target_bir_lowering=False, debug=False,
                   num_devices=N_CORES)
    dt = mybir.dt
    x = nc.dram_tensor("x", [128, N_LOC // 128 * W_IN], dt.uint8,
                       kind="ExternalInput").ap()
    luts = nc.dram_tensor("luts", [128, 32], dt.float16,
                          kind="ExternalInput").ap()
    sel2 = nc.dram_tensor("sel2", [2, 128], dt.float16,
                          kind="ExternalInput").ap()
    y = nc.dram_tensor("y", [128, N_LOC // 128 * W_OUT], dt.int32,
                       kind="ExternalOutput").ap()
    ct_scratch = nc.dram_tensor("ct_scratch", [2, 128, GROUP_BLKS * 128],
                                dt.float16).ap()

    with tile.TileContext(nc) as tc:
        with (
            tc.tile_pool(name="const", bufs=1) as constp,
            tc.tile_pool(name="xin", bufs=3) as xinp,
            tc.tile_pool(name="scan", bufs=3) as scanp,
            tc.tile_pool(name="sel", bufs=2) as selp,
            tc.tile_pool(name="ct", bufs=2) as ctp,
            tc.tile_pool(name="stg", bufs=2) as stgp,
            tc.tile_pool(name="bc", bufs=4) as bcp,
            tc.tile_pool(name="w", bufs=4) as wp,
            tc.tile_pool(name="scr", bufs=2) as scrp,
            tc.tile_pool(name="out", bufs=3) as outp,
            tc.tile_pool(name="pct", bufs=1, space="PSUM") as pctp,
            tc.tile_pool(name="pbc", bufs=2, space="PSUM") as pbcp,
            tc.tile_pool(name="pgd", bufs=2, space="PSUM") as pgdp,
        ):
            t_luts = constp.tile([128, 32], dt.float16)
            nc.sync.dma_start(out=t_luts[:], in_=luts[:])
            t_iota32 = constp.tile([128, 1], dt.int32)
            nc.gpsimd.iota(t_iota32[:], pattern=[[0, 1]], base=0,
                           channel_multiplier=1)
            t_iota64i = constp.tile([128, 1], dt.int32)
            nc.vector.tensor_scalar(out=t_iota64i[:], in0=t_iota32[:],
                                    scalar1=63, scalar2=None,
                                    op0=mybir.AluOpType.bitwise_and)
            t_iota64 = constp.tile([128, 1], dt.float32)
            nc.vector.tensor_copy(t_iota64[:], t_iota64i[:])
            t_ident = constp.tile([128, 128], dt.float32)
            from concourse.masks import make_identity
            make_identity(nc, t_ident[:])
            # sel2 [2, 128]: row0 -> partitions 0-63, row1 -> partitions 64-127
            t_sel2 = constp.tile([2, 128], dt.float16)
            nc.sync.dma_start(out=t_sel2[:], in_=sel2[:])

            for g in range(N_GROUPS):
                # --- scan + transpose + selector tiles for this group ---
                t_ct_ps = pctp.tile([128, GROUP_BLKS * 128], dt.float32)
                t_s6w = selp.tile([128, GROUP_BLKS * EPP * 4], dt.float16,
                                  tag="s6")
                t_s7w = selp.tile([128, GROUP_BLKS * EPP * 4], dt.float16,
                                  tag="s7")
                t_s67w = selp.tile([128, GROUP_BLKS * EPP * 4], dt.float16,
                                   tag="s67")
                for bb in range(GROUP_BLKS):
                    b = g * GROUP_BLKS + bb
                    t_x = xinp.tile([128, EPP * 8], dt.uint8)
                    nc.sync.dma_start(
                        out=t_x[:],
                        in_=x[:, b * EPP * 8:(b + 1) * EPP * 8])
                    x3 = t_x[:].rearrange("p (e k) -> p e k", k=8)
                    t_f1 = scanp.tile([128, EPP * 4], dt.int16, tag="f1")
                    f13 = t_f1[:].rearrange("p (e k) -> p e k", k=4)
                    nc.vector.scalar_tensor_tensor(
                        out=f13[:, :, :], in0=x3[:, :, 1::2], scalar=2.0,
                        in1=x3[:, :, 0::2], op0=mybir.AluOpType.mult,
                        op1=mybir.AluOpType.add)
                    t_f2 = scanp.tile([128, EPP * 2], dt.int16, tag="f2")
                    f23 = t_f2[:].rearrange("p (e k) -> p e k", k=2)
                    nc.vector.scalar_tensor_tensor(
                        out=f23[:, :, :], in0=f13[:, :, 1::2], scalar=4.0,
                        in1=f13[:, :, 0::2], op0=mybir.AluOpType.mult,
                        op1=mybir.AluOpType.add)
                    # f2[:,:,0] = bits0-3 value, f2[:,:,1] = bits4-7 value
                    t_c8 = scanp.tile([128, EPP], dt.int16, tag="c8")
                    nc.vector.scalar_tensor_tensor(
                        out=t_c8[:], in0=f23[:, :, 1], scalar=16.0,
                        in1=f23[:, :, 0], op0=mybir.AluOpType.mult,
                        op1=mybir.AluOpType.add)
                    t_c67 = scanp.tile([128, EPP], dt.int16, tag="c67")
                    nc.vector.scalar_tensor_tensor(
                        out=t_c67[:], in0=x3[:, :, 6], scalar=-64.0,
                        in1=t_c8[:], op0=mybir.AluOpType.mult,
                        op1=mybir.AluOpType.add)
                    t_c6 = scanp.tile([128, EPP], dt.float32, tag="c6")
                    nc.vector.scalar_tensor_tensor(
                        out=t_c6[:], in0=x3[:, :, 7], scalar=-128.0,
                        in1=t_c67[:], op0=mybir.AluOpType.mult,
                        op1=mybir.AluOpType.add)
                    # selector tiles (w-replicated x4): cols bb*512 + e*4 + w
                    def selview(t):
                        return t[:, bb * EPP * 4:(bb + 1) * EPP * 4]
                    def xbit(j):
                        v = x3[:, :, j]
                        return bass.AP(tensor=v.tensor, offset=v.offset,
                                       ap=v.ap + [[0, 4]])
                    nc.vector.tensor_scalar(
                        out=selview(t_s6w).rearrange("p (e k) -> p e k", k=4),
                        in0=xbit(6), scalar1=1.0, scalar2=None,
                        op0=mybir.AluOpType.mult)
                    nc.vector.tensor_scalar(
                        out=selview(t_s7w).rearrange("p (e k) -> p e k", k=4),
                        in0=xbit(7), scalar1=1.0, scalar2=None,
                        op0=mybir.AluOpType.mult)
                    nc.vector.tensor_tensor(
                        out=selview(t_s67w), in0=selview(t_s6w),
                        in1=selview(t_s7w), op=mybir.AluOpType.mult)
                    # transpose c6 into group psum columns
                    nc.tensor.transpose(
                        t_ct_ps[:, bb * 128:(bb + 1) * 128],
                        t_c6[:], t_ident[:])
                t_ct = ctp.tile([128, GROUP_BLKS * 128], dt.float16)
                nc.scalar.copy(t_ct[:], t_ct_ps[:])
                nc.sync.dma_start(out=ct_scratch[g % 2], in_=t_ct[:])

                # --- pair loop ---
                t_pg = None
                t_stage = None
                STG_PAIRS = 16
                for jp in range(PAIRS):
                    jd = jp % DRAIN_PAIRS
                    if jd == 0:
                        t_pg = pgdp.tile([128, DRAIN_PAIRS * 128], dt.float32)
                    if jp % STG_PAIRS == 0:
                        # load 16 pairs: stage[r, k*512+f] = ct[i0+2k+r, f]
                        t_stage = stgp.tile([2, STG_PAIRS * 512], dt.float16)
                        cs = ct_scratch[g % 2]
                        src = bass.AP(
                            tensor=cs.tensor,
                            offset=cs.offset + (2 * jp) * 512,
                            ap=[[512, 2], [1024, STG_PAIRS], [1, 512]])
                        nc.sync.dma_start(out=t_stage[:].rearrange(
                            "p (k f) -> p k f", f=512), in_=src)
                        # odd rows again at partition 0 for gpsimd bcast
                        t_stage1 = stgp.tile([1, STG_PAIRS * 512], dt.float16,
                                             tag="s1")
                        src1 = bass.AP(
                            tensor=cs.tensor,
                            offset=cs.offset + (2 * jp + 1) * 512,
                            ap=[[512, 1], [1024, STG_PAIRS], [1, 512]])
                        nc.sync.dma_start(out=t_stage1[:].rearrange(
                            "p (k f) -> p k f", f=512), in_=src1)
                    # broadcast pair of code rows
                    ks = (jp % STG_PAIRS) * 512
                    t_bc = bcp.tile([128, 512], dt.float16)
                    if False:
                        # gpsimd path: two full broadcasts, half is_eq each
                        t_bcB = bcp.tile([128, 512], dt.float16, tag="bcB")
                        nc.gpsimd.partition_broadcast(
                            t_bc[:], t_stage[0:1, ks:ks + 512])
                        nc.gpsimd.partition_broadcast(
                            t_bcB[:], t_stage1[0:1, ks:ks + 512])
                        t_w = wp.tile([128, 512], dt.float16)
                        nc.vector.tensor_scalar(
                            out=t_w[0:64, :], in0=t_bc[0:64, :],
                            scalar1=t_iota64[0:64], scalar2=None,
                            op0=mybir.AluOpType.is_equal)
                        nc.vector.tensor_scalar(
                            out=t_w[64:128, :], in0=t_bcB[64:128, :],
                            scalar1=t_iota64[64:128], scalar2=None,
                            op0=mybir.AluOpType.is_equal)
                    else:
                        t_bc_ps = pbcp.tile([128, 512], dt.float32)
                        nc.tensor.matmul(t_bc_ps[:], t_sel2[:],
                                         t_stage[0:2, ks:ks + 512],
                                         start=True, stop=True,
                                         tile_position=(0, 0))
                        if jp % 2 == 0:
                            nc.scalar.copy(t_bc[:], t_bc_ps[:])
                        else:
                            nc.vector.tensor_copy(t_bc[:], t_bc_ps[:])
                        t_w = wp.tile([128, 512], dt.float16)
                        nc.vector.tensor_scalar(
                            out=t_w[:], in0=t_bc[:], scalar1=t_iota64[:],
                            scalar2=None, op0=mybir.AluOpType.is_equal)
                    # gather: per block, one-hot chunk stationary x luts2
                    for bb in range(GROUP_BLKS):
                        nc.tensor.matmul(
                            t_pg[:, jd * 128 + bb * 32:jd * 128 + bb * 32 + 32],
                            t_w[:, bb * 128:(bb + 1) * 128],
                            t_luts[:],
                            start=True, stop=True)
                    if jd == DRAIN_PAIRS - 1:
                        # recombine 8 pairs: psum cols j*128 + bb*32 + tbl*8
                        #   + r*4 + w ; view dims (bb, j, rw)
                        i0 = (jp - jd) * 2      # first row index of drain
                        P = t_pg[:]
                        def tblview(t):
                            return bass.AP(
                                tensor=P.tensor, offset=P.offset + t * 8,
                                ap=[P.ap[0], [32, 4], [128, 8], [1, 8]])
                        def sview(t):
                            base = t[:]
                            return bass.AP(
                                tensor=base.tensor,
                                offset=base.offset + i0 * 4,
                                ap=[base.ap[0], [EPP * 4, 4], [8, 8], [1, 8]])
                        t_u1 = scrp.tile([128, 256], dt.float32, tag="u1")
                        t_u2 = scrp.tile([128, 256], dt.float32, tag="u2")
                        t_u3 = scrp.tile([128, 256], dt.float32, tag="u3")
                        scr3 = lambda t: t[:].rearrange(
                            "p (a b c) -> p a b c", a=4, b=8)
                        nc.vector.tensor_tensor(
                            out=scr3(t_u1), in0=tblview(1), in1=sview(t_s6w),
                            op=mybir.AluOpType.mult)
                        nc.vector.tensor_tensor(
                            out=scr3(t_u2), in0=tblview(2), in1=sview(t_s7w),
                            op=mybir.AluOpType.mult)
                        nc.vector.tensor_tensor(
                            out=scr3(t_u3), in0=tblview(3), in1=sview(t_s67w),
                            op=mybir.AluOpType.mult)
                        t_a1 = scrp.tile([128, 256], dt.float32, tag="a1")
                        nc.vector.tensor_tensor(
                            out=scr3(t_a1), in0=tblview(0), in1=scr3(t_u1),
                            op=mybir.AluOpType.add)
                        t_a2 = scrp.tile([128, 256], dt.float32, tag="a2")
                        nc.vector.tensor_tensor(
                            out=scr3(t_a2), in0=scr3(t_u2), in1=scr3(t_u3),
                            op=mybir.AluOpType.add)
                        t_o = outp.tile([128, 256], dt.int32)
                        nc.vector.tensor_tensor(
                            out=scr3(t_o), in0=scr3(t_a1), in1=scr3(t_a2),
                            op=mybir.AluOpType.add)
                        # dma out: o cols (bb, j, rw) -> y col b*512 + i0*4
                        #   + (j*8 + r*4 + w);  (j, rw) contiguous 64
                        yv = bass.AP(
                            tensor=y.tensor,
                            offset=y.offset + g * GROUP_BLKS * 512 + i0 * 4,
                            ap=[y.ap[0], [512, 4], [1, 64]])
                        ov = t_o[:].rearrange("p (a b) -> p a b", a=4)
                        nc.sync.dma_start(out=yv, in_=ov)
    nc.compile()
    return nc


_CACHE = {}


def kernel(x: np.ndarray, patterns: np.ndarray, results: np.ndarray) -> np.ndarray:
    import jax
    from jax.sharding import Mesh, PartitionSpec, NamedSharding
    from jax.experimental.shard_map import shard_map
    from concourse import mybir
    from concourse.bass2jax import (_bass_exec_p, install_neuronx_cc_hook,
                                    partition_id_tensor)

    x = np.asarray(x)
    patterns = np.asarray(patterns)
    results = np.asarray(results)
    rhs_luts = _build_luts(patterns, results)

    if "nc" not in _CACHE:
        _CACHE["nc"] = _build_kernel()
    nc = _CACHE["nc"]

    install_neuronx_cc_hook()
    partition_name = nc.partition_id_tensor.name if nc.partition_id_tensor else None
    in_names, out_names, out_avals, zero_outs = [], [], [], []
    for alloc in nc.m.functions[0].allocations:
        if not isinstance(alloc, mybir.MemoryLocationSet):
            continue
        name = alloc.memorylocations[0].name
        if alloc.kind == "ExternalInput":
            if name != partition_name:
                in_names.append(name)
        elif alloc.kind == "ExternalOutput":
            out_names.append(name)
            shape = tuple(alloc.tensor_shape)
            dtype = mybir.dt.np(alloc.dtype)
            out_avals.append(jax.core.ShapedArray(shape, dtype))
            zero_outs.append(np.zeros(shape, dtype))
    n_params = len(in_names)
    n_outs = len(out_avals)
    all_in_names = in_names + out_names + ([partition_name] if partition_name else [])

    def _body(*args):
        operands = list(args)
        if partition_name is not None:
            operands.append(partition_id_tensor())
        outs = _bass_exec_p.bind(
            *operands, out_avals=tuple(out_avals), in_names=tuple(all_in_names),
            out_names=tuple(out_names), lowering_input_output_aliases=(),
            sim_require_finite=False, sim_require_nnan=False, nc=nc)
        return tuple(outs)

    devices = jax.devices()[:N_CORES]
    mesh = Mesh(np.asarray(devices), ("core",))
    shard = NamedSharding(mesh, PartitionSpec("core"))
    fn = jax.jit(
        shard_map(_body, mesh=mesh,
                  in_specs=(PartitionSpec("core"),) * (n_params + n_outs),
                  out_specs=(PartitionSpec("core"),) * n_outs,
                  check_rep=False),
        keep_unused=True)

    # Per-core input planes; element n_loc = b*16384 + p*128 + i.
    # x values are 0/1 int32 little-endian: byte 0 of each word is the bit.
    xb = x.view(np.uint8).reshape(N, W_IN, 4)[:, :, 0]         # [N, 8] uint8
    xc = xb.reshape(N_CORES, N_BLKS, 128, EPP * W_IN)          # [c, b, p, 128*8]
    x_in = np.ascontiguousarray(xc.transpose(0, 2, 1, 3)).reshape(
        N_CORES * 128, N_BLKS * EPP * W_IN)
    luts_in = np.broadcast_to(rhs_luts, (N_CORES, 128, 32)).reshape(
        N_CORES * 128, 32)
    sel2_np = np.zeros((2, 128), np.float16)
    sel2_np[0, 0:64] = 1.0
    sel2_np[1, 64:128] = 1.0
    sel2_in = np.broadcast_to(sel2_np, (N_CORES, 2, 128)).reshape(N_CORES * 2, 128)
    arrays = {"x": x_in, "luts": np.ascontiguousarray(luts_in),
              "sel2": np.ascontiguousarray(sel2_in)}
    args = [jax.device_put(arrays[nm], shard) for nm in in_names]
    args += [jax.device_put(
        np.zeros((N_CORES * z.shape[0], *z.shape[1:]), z.dtype), shard)
        for z in zero_outs]
    out_arrs = fn(*args)
    yi = out_names.index("y")
    yv = np.asarray(out_arrs[yi]).reshape(N_CORES, 128, N_BLKS, EPP * W_OUT)
    # invert layout: [c, p, b, 128*4] -> [c, b, p, i, 4] -> n
    y_full = yv.transpose(0, 2, 1, 3).reshape(N, W_OUT)
    return y_full.astype(np.int32)


# revision 15
# speedup vs baseline: 1.4532x; 1.1797x over previous
"""CBIndirectionLookup Trainium2 kernel (v15: 2-bit selector fold, 64-wide
one-hot, row-paired broadcast/one-hot/gather, batched recombine).

Problem: x [N=2097152, 8] int32 bit-vectors; patterns [256, 8] (unique bit rows);
results [256, 4] int32. Output: results[argmax(all(x==patterns))] -> [N, 4] int32.

Math: c8 = sum_j x_j 2^j = c6 + 64 b6 + 128 b7.  With Moebius tables over
(b6, b7):
    B   = T[c6],             D6  = T[c6+64]  - T[c6]
    D7  = T[c6+128] - T[c6], D67 = T[c6+192] - T[c6+128] - T[c6+64] + T[c6]
    y = B + b6*D6 + b7*D7 + (b6 b7)*D67        (all |.| <= 1998: fp16-exact)

Per core (262144 elems; 4 groups of 4 blocks; block = [128p, 128i]):
 1. DVE Horner on uint8 bytes -> c8 int16 -> c6 fp32; selector tiles
    s6w/s7w/s67w [128, 2048] fp16 (w-replicated x4).
 2. PE transpose c6 -> ct [128, 512] fp16 rows per group (via psum + ACT copy).
 3. Per row-PAIR (64/group): one matmul with lhsT=sel2 [2,128] broadcasts row
    2j to partitions 0-63 and row 2j+1 to 64-127; ACT-copy psum->sbuf fp16;
    one is_equal vs iota64 -> one-hot pair [128, 512] fp16.
 4. Gather: per block bb, matmul(lhsT=onehot chunk [128,128], rhs=luts2
    [128, 32]) -> psum [128elem, 32] = (tbl, r, w) fetches, element-major.
 5. Every 8 pairs: 6 batched DVE ops recombine y = B + s6 D6 + s7 D7 + s67 D67
    -> int32, one DMA out per 2-bank drain.
"""
import sys
sys.path.insert(0, "/opt/trn_rl_repo")

import numpy as np

N = 2_097_152
W_IN = 8
W_OUT = 4
N_CORES = 8
N_LOC = N // N_CORES            # 262144 elements per core
BLK_ELEMS = 16384               # elements per block: [128, 128] layout
N_BLKS = N_LOC // BLK_ELEMS     # 16
EPP = BLK_ELEMS // 128          # 128 elements per partition per block
GROUP_BLKS = 4                  # blocks per code-transpose group
N_GROUPS = N_BLKS // GROUP_BLKS
PAIRS = 64                      # row pairs per group
DRAIN_PAIRS = 8                 # pairs per psum drain (2 banks)


def _build_luts(patterns: np.ndarray, results: np.ndarray):
    """Fold tables into luts2 [128, 32] fp16: block-diag 2x luts16 [64, 16].

    luts16[c6, tbl*4+w] with tbl in {B, D6, D7, D67}; luts2 col layout
    tbl*8 + r*4 + w (r = which row of the pair / diag block)."""
    pat2idx = {}
    for p in range(patterns.shape[0]):
        pat2idx[tuple(int(v) for v in patterns[p])] = p
    T = np.zeros((256, W_OUT), np.float64)
    for c in range(256):
        bits = [(c >> j) & 1 for j in range(8)]
        T[c] = results[pat2idx[tuple(bits)]]
    c6 = np.arange(64)
    B = T[c6]
    D6 = T[c6 + 64] - T[c6]
    D7 = T[c6 + 128] - T[c6]
    D67 = T[c6 + 192] - T[c6 + 128] - T[c6 + 64] + T[c6]
    luts16 = np.stack([B, D6, D7, D67], axis=1)        # [64, 4tbl, 4w]
    luts2 = np.zeros((128, 4, 2, 4), np.float64)       # [row, tbl, r, w]
    luts2[0:64, :, 0, :] = luts16
    luts2[64:128, :, 1, :] = luts16
    out = luts2.reshape(128, 32).astype(np.float16)
    assert np.array_equal(out.astype(np.float64), luts2.reshape(128, 32))
    return out


def _build_kernel():
    import concourse.bass as bass
    import concourse.bacc as bacc
    import concourse.tile as tile
    from concourse import mybir

    nc = bacc.Bacc("TRN2", target_bir_lowering=False, debug=False,
                   num_devices=N_CORES)
    dt = mybir.dt
    x = nc.dram_tensor("x", [128, N_LOC // 128 * W_IN], dt.uint8,
                       kind="ExternalInput").ap()
    luts = nc.dram_tensor("luts", [128, 32], dt.float16,
                          kind="ExternalInput").ap()
    sel2 = nc.dram_tensor("sel2", [2, 128], dt.float16,
                          kind="ExternalInput").ap()
    y = nc.dram_tensor("y", [128, N_LOC // 128 * W_OUT], dt.int32,
                       kind="ExternalOutput").ap()
    ct_scratch = nc.dram_tensor("ct_scratch", [2, 128, GROUP_BLKS * 128],
                                dt.float16).ap()

    with tile.TileContext(nc) as tc:
        with (
            tc.tile_pool(name="const", bufs=1) as constp,
            tc.tile_pool(name="xin", bufs=3) as xinp,
            tc.tile_pool(name="scan", bufs=3) as scanp,
            tc.tile_pool(name="sel", bufs=2) as selp,
            tc.tile_pool(name="ct", bufs=2) as ctp,
            tc.tile_pool(name="stg", bufs=2) as stgp,
            tc.tile_pool(name="bc", bufs=4) as bcp,
            tc.tile_pool(name="w", bufs=4) as wp,
            tc.tile_pool(name="scr", bufs=2) as scrp,
            tc.tile_pool(name="out", bufs=3) as outp,
            tc.tile_pool(name="pct", bufs=1, space="PSUM") as pctp,
            tc.tile_pool(name="pbc", bufs=2, space="PSUM") as pbcp,
            tc.tile_pool(name="pgd", bufs=2, space="PSUM") as pgdp,
        ):
            t_luts = constp.tile([128, 32], dt.float16)
            nc.sync.dma_start(out=t_luts[:], in_=luts[:])
            t_iota32 = constp.tile([128, 1], dt.int32)
            nc.gpsimd.iota(t_iota32[:], pattern=[[0, 1]], base=0,
                           channel_multiplier=1)
            t_iota64i = constp.tile([128, 1], dt.int32)
            nc.vector.tensor_scalar(out=t_iota64i[:], in0=t_iota32[:],
                                    scalar1=63, scalar2=None,
                                    op0=mybir.AluOpType.bitwise_and)
            t_iota64 = constp.tile([128, 1], dt.float32)
            nc.vector.tensor_copy(t_iota64[:], t_iota64i[:])
            t_ident = constp.tile([128, 128], dt.float32)
            from concourse.masks import make_identity
            make_identity(nc, t_ident[:])
            # sel2 [2, 128]: row0 -> partitions 0-63, row1 -> partitions 64-127
            t_sel2 = constp.tile([2, 128], dt.float16)
            nc.sync.dma_start(out=t_sel2[:], in_=sel2[:])

            for g in range(N_GROUPS):
                # --- scan + transpose + selector tiles for this group ---
                t_ct_ps = pctp.tile([128, GROUP_BLKS * 128], dt.float32)
                t_s6w = selp.tile([128, GROUP_BLKS * EPP * 4], dt.float16,
                                  tag="s6")
                t_s7w = selp.tile([128, GROUP_BLKS * EPP * 4], dt.float16,
                                  tag="s7")
                t_s67w = selp.tile([128, GROUP_BLKS * EPP * 4], dt.float16,
                                   tag="s67")
                for bb in range(GROUP_BLKS):
                    b = g * GROUP_BLKS + bb
                    t_x = xinp.tile([128, EPP * 8], dt.uint8)
                    nc.sync.dma_start(
                        out=t_x[:],
                        in_=x[:, b * EPP * 8:(b + 1) * EPP * 8])
                    x3 = t_x[:].rearrange("p (e k) -> p e k", k=8)
                    t_f1 = scanp.tile([128, EPP * 4], dt.int16, tag="f1")
                    f13 = t_f1[:].rearrange("p (e k) -> p e k", k=4)
                    nc.vector.scalar_tensor_tensor(
                        out=f13[:, :, :], in0=x3[:, :, 1::2], scalar=2.0,
                        in1=x3[:, :, 0::2], op0=mybir.AluOpType.mult,
                        op1=mybir.AluOpType.add)
                    t_f2 = scanp.tile([128, EPP * 2], dt.int16, tag="f2")
                    f23 = t_f2[:].rearrange("p (e k) -> p e k", k=2)
                    nc.vector.scalar_tensor_tensor(
                        out=f23[:, :, :], in0=f13[:, :, 1::2], scalar=4.0,
                        in1=f13[:, :, 0::2], op0=mybir.AluOpType.mult,
                        op1=mybir.AluOpType.add)
                    # f2[:,:,0] = bits0-3 value, f2[:,:,1] = bits4-7 value
                    t_c8 = scanp.tile([128, EPP], dt.int16, tag="c8")
                    nc.vector.scalar_tensor_tensor(
                        out=t_c8[:], in0=f23[:, :, 1], scalar=16.0,
                        in1=f23[:, :, 0], op0=mybir.AluOpType.mult,
                        op1=mybir.AluOpType.add)
                    t_c67 = scanp.tile([128, EPP], dt.int16, tag="c67")
                    nc.vector.scalar_tensor_tensor(
                        out=t_c67[:], in0=x3[:, :, 6], scalar=-64.0,
                        in1=t_c8[:], op0=mybir.AluOpType.mult,
                        op1=mybir.AluOpType.add)
                    t_c6 = scanp.tile([128, EPP], dt.float32, tag="c6")
                    nc.vector.scalar_tensor_tensor(
                        out=t_c6[:], in0=x3[:, :, 7], scalar=-128.0,
                        in1=t_c67[:], op0=mybir.AluOpType.mult,
                        op1=mybir.AluOpType.add)
                    # selector tiles (w-replicated x4): cols bb*512 + e*4 + w
                    def selview(t):
                        return t[:, bb * EPP * 4:(bb + 1) * EPP * 4]
                    def xbit(j):
                        v = x3[:, :, j]
                        return bass.AP(tensor=v.tensor, offset=v.offset,
                                       ap=v.ap + [[0, 4]])
                    nc.vector.tensor_scalar(
                        out=selview(t_s6w).rearrange("p (e k) -> p e k", k=4),
                        in0=xbit(6), scalar1=1.0, scalar2=None,
                        op0=mybir.AluOpType.mult)
                    nc.vector.tensor_scalar(
                        out=selview(t_s7w).rearrange("p (e k) -> p e k", k=4),
                        in0=xbit(7), scalar1=1.0, scalar2=None,
                        op0=mybir.AluOpType.mult)
                    nc.vector.tensor_tensor(
                        out=selview(t_s67w), in0=selview(t_s6w),
                        in1=selview(t_s7w), op=mybir.AluOpType.mult)
                    # transpose c6 into group psum columns
                    nc.tensor.transpose(
                        t_ct_ps[:, bb * 128:(bb + 1) * 128],
                        t_c6[:], t_ident[:])
                t_ct = ctp.tile([128, GROUP_BLKS * 128], dt.float16)
                nc.scalar.copy(t_ct[:], t_ct_ps[:])
                nc.sync.dma_start(out=ct_scratch[g % 2], in_=t_ct[:])

                # --- pair loop ---
                t_pg = None
                t_stage = None
                STG_PAIRS = 16
                for jp in range(PAIRS):
                    jd = jp % DRAIN_PAIRS
                    if jd == 0:
                        t_pg = pgdp.tile([128, DRAIN_PAIRS * 128], dt.float32)
                    if jp % STG_PAIRS == 0:
                        # load 16 pairs: stage[r, k*512+f] = ct[i0+2k+r, f]
                        t_stage = stgp.tile([2, STG_PAIRS * 512], dt.float16)
                        cs = ct_scratch[g % 2]
                        src = bass.AP(
                            tensor=cs.tensor,
                            offset=cs.offset + (2 * jp) * 512,
                            ap=[[512, 2], [1024, STG_PAIRS], [1, 512]])
                        nc.sync.dma_start(out=t_stage[:].rearrange(
                            "p (k f) -> p k f", f=512), in_=src)

                    # broadcast pair of code rows
                    ks = (jp % STG_PAIRS) * 512
                    t_bc = bcp.tile([128, 512], dt.float16)
                    if False:
                        # gpsimd path: two full broadcasts, half is_eq each
                        t_bcB = bcp.tile([128, 512], dt.float16, tag="bcB")
                        nc.gpsimd.partition_broadcast(
                            t_bc[:], t_stage[0:1, ks:ks + 512])
                        nc.gpsimd.partition_broadcast(
                            t_bcB[:], t_stage1[0:1, ks:ks + 512])
                        t_w = wp.tile([128, 512], dt.float16)
                        nc.vector.tensor_scalar(
                            out=t_w[0:64, :], in0=t_bc[0:64, :],
                            scalar1=t_iota64[0:64], scalar2=None,
                            op0=mybir.AluOpType.is_equal)
                        nc.vector.tensor_scalar(
                            out=t_w[64:128, :], in0=t_bcB[64:128, :],
                            scalar1=t_iota64[64:128], scalar2=None,
                            op0=mybir.AluOpType.is_equal)
                    else:
                        t_bc_ps = pbcp.tile([128, 512], dt.float32)
                        nc.tensor.matmul(t_bc_ps[:], t_sel2[:],
                                         t_stage[0:2, ks:ks + 512],
                                         start=True, stop=True,
                                         tile_position=(0, 0))
                        nc.scalar.copy(t_bc[:], t_bc_ps[:])
                        t_w = wp.tile([128, 512], dt.float16)
                        nc.vector.tensor_scalar(
                            out=t_w[:], in0=t_bc[:], scalar1=t_iota64[:],
                            scalar2=None, op0=mybir.AluOpType.is_equal)
                    # gather: per block, one-hot chunk stationary x luts2
                    for bb in range(GROUP_BLKS):
                        nc.tensor.matmul(
                            t_pg[:, jd * 128 + bb * 32:jd * 128 + bb * 32 + 32],
                            t_w[:, bb * 128:(bb + 1) * 128],
                            t_luts[:],
                            start=True, stop=True)
                    if jd == DRAIN_PAIRS - 1:
                        # recombine 8 pairs: psum cols j*128 + bb*32 + tbl*8
                        #   + r*4 + w ; view dims (bb, j, rw)
                        i0 = (jp - jd) * 2      # first row index of drain
                        P = t_pg[:]
                        def tblview(t):
                            return bass.AP(
                                tensor=P.tensor, offset=P.offset + t * 8,
                                ap=[P.ap[0], [32, 4], [128, 8], [1, 8]])
                        def sview(t):
                            base = t[:]
                            return bass.AP(
                                tensor=base.tensor,
                                offset=base.offset + i0 * 4,
                                ap=[base.ap[0], [EPP * 4, 4], [8, 8], [1, 8]])
                        t_u1 = scrp.tile([128, 256], dt.float32, tag="u1")
                        t_u2 = scrp.tile([128, 256], dt.float32, tag="u2")
                        t_u3 = scrp.tile([128, 256], dt.float32, tag="u3")
                        scr3 = lambda t: t[:].rearrange(
                            "p (a b c) -> p a b c", a=4, b=8)
                        nc.vector.tensor_tensor(
                            out=scr3(t_u1), in0=tblview(1), in1=sview(t_s6w),
                            op=mybir.AluOpType.mult)
                        nc.vector.tensor_tensor(
                            out=scr3(t_u2), in0=tblview(2), in1=sview(t_s7w),
                            op=mybir.AluOpType.mult)
                        nc.vector.tensor_tensor(
                            out=scr3(t_u3), in0=tblview(3), in1=sview(t_s67w),
                            op=mybir.AluOpType.mult)
                        t_a1 = scrp.tile([128, 256], dt.float32, tag="a1")
                        nc.vector.tensor_tensor(
                            out=scr3(t_a1), in0=tblview(0), in1=scr3(t_u1),
                            op=mybir.AluOpType.add)
                        t_a2 = scrp.tile([128, 256], dt.float32, tag="a2")
                        nc.vector.tensor_tensor(
                            out=scr3(t_a2), in0=scr3(t_u2), in1=scr3(t_u3),
                            op=mybir.AluOpType.add)
                        t_o = outp.tile([128, 256], dt.int32)
                        nc.vector.tensor_tensor(
                            out=scr3(t_o), in0=scr3(t_a1), in1=scr3(t_a2),
                            op=mybir.AluOpType.add)
                        # dma out: o cols (bb, j, rw) -> y col b*512 + i0*4
                        #   + (j*8 + r*4 + w);  (j, rw) contiguous 64
                        yv = bass.AP(
                            tensor=y.tensor,
                            offset=y.offset + g * GROUP_BLKS * 512 + i0 * 4,
                            ap=[y.ap[0], [512, 4], [1, 64]])
                        ov = t_o[:].rearrange("p (a b) -> p a b", a=4)
                        nc.sync.dma_start(out=yv, in_=ov)
    nc.compile()
    return nc


_CACHE = {}


def kernel(x: np.ndarray, patterns: np.ndarray, results: np.ndarray) -> np.ndarray:
    import jax
    from jax.sharding import Mesh, PartitionSpec, NamedSharding
    from jax.experimental.shard_map import shard_map
    from concourse import mybir
    from concourse.bass2jax import (_bass_exec_p, install_neuronx_cc_hook,
                                    partition_id_tensor)

    x = np.asarray(x)
    patterns = np.asarray(patterns)
    results = np.asarray(results)
    rhs_luts = _build_luts(patterns, results)

    if "nc" not in _CACHE:
        _CACHE["nc"] = _build_kernel()
    nc = _CACHE["nc"]

    install_neuronx_cc_hook()
    partition_name = nc.partition_id_tensor.name if nc.partition_id_tensor else None
    in_names, out_names, out_avals, zero_outs = [], [], [], []
    for alloc in nc.m.functions[0].allocations:
        if not isinstance(alloc, mybir.MemoryLocationSet):
            continue
        name = alloc.memorylocations[0].name
        if alloc.kind == "ExternalInput":
            if name != partition_name:
                in_names.append(name)
        elif alloc.kind == "ExternalOutput":
            out_names.append(name)
            shape = tuple(alloc.tensor_shape)
            dtype = mybir.dt.np(alloc.dtype)
            out_avals.append(jax.core.ShapedArray(shape, dtype))
            zero_outs.append(np.zeros(shape, dtype))
    n_params = len(in_names)
    n_outs = len(out_avals)
    all_in_names = in_names + out_names + ([partition_name] if partition_name else [])

    def _body(*args):
        operands = list(args)
        if partition_name is not None:
            operands.append(partition_id_tensor())
        outs = _bass_exec_p.bind(
            *operands, out_avals=tuple(out_avals), in_names=tuple(all_in_names),
            out_names=tuple(out_names), lowering_input_output_aliases=(),
            sim_require_finite=False, sim_require_nnan=False, nc=nc)
        return tuple(outs)

    devices = jax.devices()[:N_CORES]
    mesh = Mesh(np.asarray(devices), ("core",))
    shard = NamedSharding(mesh, PartitionSpec("core"))
    fn = jax.jit(
        shard_map(_body, mesh=mesh,
                  in_specs=(PartitionSpec("core"),) * (n_params + n_outs),
                  out_specs=(PartitionSpec("core"),) * n_outs,
                  check_rep=False),
        keep_unused=True)

    # Per-core input planes; element n_loc = b*16384 + p*128 + i.
    # x values are 0/1 int32 little-endian: byte 0 of each word is the bit.
    xb = x.view(np.uint8).reshape(N, W_IN, 4)[:, :, 0]         # [N, 8] uint8
    xc = xb.reshape(N_CORES, N_BLKS, 128, EPP * W_IN)          # [c, b, p, 128*8]
    x_in = np.ascontiguousarray(xc.transpose(0, 2, 1, 3)).reshape(
        N_CORES * 128, N_BLKS * EPP * W_IN)
    luts_in = np.broadcast_to(rhs_luts, (N_CORES, 128, 32)).reshape(
        N_CORES * 128, 32)
    sel2_np = np.zeros((2, 128), np.float16)
    sel2_np[0, 0:64] = 1.0
    sel2_np[1, 64:128] = 1.0
    sel2_in = np.broadcast_to(sel2_np, (N_CORES, 2, 128)).reshape(N_CORES * 2, 128)
    arrays = {"x": x_in, "luts": np.ascontiguousarray(luts_in),
              "sel2": np.ascontiguousarray(sel2_in)}
    args = [jax.device_put(arrays[nm], shard) for nm in in_names]
    args += [jax.device_put(
        np.zeros((N_CORES * z.shape[0], *z.shape[1:]), z.dtype), shard)
        for z in zero_outs]
    out_arrs = fn(*args)
    yi = out_names.index("y")
    yv = np.asarray(out_arrs[yi]).reshape(N_CORES, 128, N_BLKS, EPP * W_OUT)
    # invert layout: [c, p, b, 128*4] -> [c, b, p, i, 4] -> n
    y_full = yv.transpose(0, 2, 1, 3).reshape(N, W_OUT)
    return y_full.astype(np.int32)
